# revision 4
# baseline (speedup 1.0000x reference)
"""Trainium2 Bass kernel for LocalWindowAttention (B=8, C=256, H=W=64, r=32).

Strategy: data-parallel over batch across 8 NeuronCores (one batch element
per core, zero collectives).  Per core everything is computed in a
"transposed" layout so that the softmax denominator (a sum over the
contraction axis m, which lives on SBUF partitions) can be computed with
cheap TensorE ones-matmuls instead of the slow VectorE:

  qT  (r=32 x N)   = (wq*scale) @ x + bq*scale   replicated to 4 row groups
  kT  (r=32 x N)   = wk @ x + bk                 replicated to 4 row groups
  vT  (m x C)      = x^T @ wv^T + bv             (ones-matmul bias trick)
  sT  (m x n)      = kT_slice^T @ qT             4x row-tiled (K=32 packing)
  E = exp(sT)      on ScalarE, PSUM -> SBUF
  colsum           = ones^T @ E   per m-chunk, 4x col-tiled into one bank
  out_u (C x n)    = vT^T @ E     accumulated over m-chunks
  out = x + gamma * out_u / colsum
"""

import numpy as np
from contextlib import ExitStack

import concourse.bass as bass
import concourse.tile as tile
from concourse import bacc, mybir, bass_utils

F32 = mybir.dt.float32
AF = mybir.ActivationFunctionType
ALU = mybir.AluOpType

B, C, HH, WW = 8, 256, 64, 64
N = HH * WW            # 4096 tokens
R = 32                 # low-rank q/k dim
NCORES = 8
NB = 512               # n-block (free dim per matmul)
NNB = N // NB          # 8
MC = 128               # m-chunk (contraction tile)
NMC = N // MC          # 32

_cache = {}


def _build_program(gamma: float):
    nc = bacc.Bacc("TRN2", debug=False, enable_asserts=True, num_devices=NCORES)
    x_d = nc.dram_tensor("x", (C, N), F32, kind="ExternalInput").ap()
    wqt_d = nc.dram_tensor("wqt", (C, R), F32, kind="ExternalInput").ap()
    wkt_d = nc.dram_tensor("wkt", (C, R), F32, kind="ExternalInput").ap()
    wvt_d = nc.dram_tensor("wvt", (C, C), F32, kind="ExternalInput").ap()
    bq4_d = nc.dram_tensor("bq4", (128, 1), F32, kind="ExternalInput").ap()
    bk4_d = nc.dram_tensor("bk4", (128, 1), F32, kind="ExternalInput").ap()
    bv_d = nc.dram_tensor("bv", (1, C), F32, kind="ExternalInput").ap()
    onesc_d = nc.dram_tensor("onesc", (128, 1), F32, kind="ExternalInput").ap()
    onesr_d = nc.dram_tensor("onesr", (1, 128), F32, kind="ExternalInput").ap()
    out_d = nc.dram_tensor("out", (C, N), F32, kind="ExternalOutput").ap()

    with tile.TileContext(nc) as tc, ExitStack() as ctx:
        consts = ctx.enter_context(tc.tile_pool(name="consts", bufs=1))
        xpool = ctx.enter_context(tc.tile_pool(name="xp", bufs=8))
        qkpool = ctx.enter_context(tc.tile_pool(name="qk", bufs=1))
        vpool = ctx.enter_context(tc.tile_pool(name="vp", bufs=1))
        exppool = ctx.enter_context(tc.tile_pool(name="ep", bufs=1))
        misc = ctx.enter_context(tc.tile_pool(name="misc", bufs=1))

        # ---- constants ----
        wqt_sb = consts.tile([128, 2, R], F32, tag="wqt")
        nc.sync.dma_start(wqt_sb[:], wqt_d.rearrange("(k p) r -> p k r", p=128))
        wkt_sb = consts.tile([128, 2, R], F32, tag="wkt")
        nc.sync.dma_start(wkt_sb[:], wkt_d.rearrange("(k p) r -> p k r", p=128))
        wvt_sb = consts.tile([128, 2, C], F32, tag="wvt")
        nc.sync.dma_start(wvt_sb[:], wvt_d.rearrange("(k p) c -> p k c", p=128))
        bq4_sb = consts.tile([128, 1], F32, tag="bq4")
        nc.sync.dma_start(bq4_sb[:], bq4_d[:])
        bk4_sb = consts.tile([128, 1], F32, tag="bk4")
        nc.sync.dma_start(bk4_sb[:], bk4_d[:])
        bv_sb = consts.tile([1, C], F32, tag="bv")
        nc.sync.dma_start(bv_sb[:], bv_d[:])
        onesc_sb = consts.tile([128, 1], F32, tag="onesc")
        nc.sync.dma_start(onesc_sb[:], onesc_d[:])
        onesr_sb = consts.tile([1, 128], F32, tag="onesr")
        nc.sync.dma_start(onesr_sb[:], onesr_d[:])

        # ---- persistent activations ----
        qt_sb = qkpool.tile([128, N], F32, tag="qt")     # q^T replicated 4x
        kt_sb = qkpool.tile([128, N], F32, tag="kt")     # k^T replicated 4x
        vt_sb = vpool.tile([128, NMC * C], F32, tag="vt")  # v^T, chunk j at [:, j*C:(j+1)*C]
        exp_sb = exppool.tile([128, NMC * NB], F32, tag="exp")  # one n-block of exp(sT)

        xt = {}

        # ================= phase 1: q/k/v projections =================
        with tc.tile_pool(name="pp", bufs=2, space=bass.MemorySpace.PSUM) as pp, \
             tc.tile_pool(name="pvp", bufs=2, space=bass.MemorySpace.PSUM) as pvp:
            for qq in range(4):
                for k in range(2):
                    t_ = xpool.tile([128, 1024], F32, tag="xt")
                    nc.sync.dma_start(
                        t_[:], x_d[k * 128:(k + 1) * 128, qq * 1024:(qq + 1) * 1024])
                    xt[(k, qq)] = t_

                # q and k projections for the two 512-col chunks of this quarter,
                # replicated into 4 col groups (-> 4 partition groups of 32)
                for half in range(2):
                    i = 2 * qq + half
                    for (w_sb, b_sb, dst) in ((wqt_sb, bq4_sb, qt_sb),
                                              (wkt_sb, bk4_sb, kt_sb)):
                        pt = pp.tile([128, NB], F32, tag="pp")
                        for g in range(4):
                            for k in range(2):
                                nc.tensor.matmul(
                                    pt[32 * g:32 * (g + 1), :],
                                    w_sb[:, k, :],
                                    xt[(k, qq)][:, half * NB:(half + 1) * NB],
                                    start=(k == 0), stop=(k == 1),
                                    tile_position=(0, 32 * g))
                        nc.vector.tensor_scalar_add(
                            dst[:, i * NB:(i + 1) * NB], pt[:], b_sb[:])

                # vT for the 8 m-chunks of this quarter
                for jj in range(8):
                    j = 8 * qq + jj
                    pv = pvp.tile([128, C], F32, tag="pv")
                    nc.tensor.matmul(pv[:], xt[(0, qq)][:, jj * 128:(jj + 1) * 128],
                                     wvt_sb[:, 0, :], start=True, stop=False)
                    nc.tensor.matmul(pv[:], xt[(1, qq)][:, jj * 128:(jj + 1) * 128],
                                     wvt_sb[:, 1, :], start=False, stop=False)
                    nc.tensor.matmul(pv[:], onesr_sb[:], bv_sb[:],
                                     start=False, stop=True)
                    nc.vector.tensor_copy(vt_sb[:, j * C:(j + 1) * C], pv[:])

        # ================= phase 2: attention =================
        with tc.tile_pool(name="ps", bufs=2, space=bass.MemorySpace.PSUM) as psp, \
             tc.tile_pool(name="po", bufs=2, space=bass.MemorySpace.PSUM) as pop, \
             tc.tile_pool(name="pc", bufs=2, space=bass.MemorySpace.PSUM) as pcp:
            for nb in range(NNB):
                po_t = [pop.tile([128, NB], F32, tag="po", name=f"po_{nb}_{i}")
                        for i in range(2)]
                pc_t = pcp.tile([128, NB], F32, tag="pc")

                def consume(tt, po_t=po_t, pc_t=pc_t, nb=nb):
                    # out_u matmuls for the 2 chunks of exp group tt
                    for g2 in range(2):
                        j = 2 * tt + g2
                        for ch in range(2):
                            nc.tensor.matmul(
                                po_t[ch][:],
                                vt_sb[:, j * C + ch * 128: j * C + (ch + 1) * 128],
                                exp_sb[:, j * NB:(j + 1) * NB],
                                start=(j == 0), stop=(j == NMC - 1))
                    # colsum matmuls: 4 chunks at a time, 4x col-tiled
                    if tt % 2 == 1:
                        t2 = tt // 2
                        for g in range(4):
                            j = 4 * t2 + g
                            nc.tensor.matmul(
                                pc_t[32 * g:32 * g + 1, :],
                                onesc_sb[:],
                                exp_sb[:, j * NB:(j + 1) * NB],
                                start=(t2 == 0), stop=(t2 == 7),
                                tile_position=(0, 32 * g))

                for t in range(16):
                    ps_t = psp.tile([128, 2 * NB], F32, tag="ps")
                    for g2 in range(2):
                        j = 2 * t + g2
                        gm = j % 4
                        nc.tensor.matmul(
                            ps_t[:, g2 * NB:(g2 + 1) * NB],
                            kt_sb[32 * gm:32 * (gm + 1), j * 128:(j + 1) * 128],
                            qt_sb[32 * gm:32 * (gm + 1), nb * NB:(nb + 1) * NB],
                            start=True, stop=True,
                            tile_position=(32 * gm, 0))
                    nc.scalar.activation(
                        exp_sb[:, 2 * t * NB:(2 * t + 2) * NB], ps_t[:], AF.Exp)
                    if t >= 1:
                        consume(t - 1)
                consume(15)

                # ---- softmax denominator combine + normalize ----
                cs_stage = misc.tile([128, NB], F32, tag="cs", name=f"cs_{nb}")
                nc.vector.tensor_copy(cs_stage[0:97, :], pc_t[0:97, :])
                cs_flat = misc.tile([1, 4 * NB], F32, tag="csf", name=f"csf_{nb}")
                for g in range(4):
                    nc.sync.dma_start(cs_flat[0:1, g * NB:(g + 1) * NB],
                                      cs_stage[32 * g:32 * g + 1, :])
                s01 = misc.tile([1, NB], F32, tag="sm", bufs=3, name=f"s01_{nb}")
                nc.vector.tensor_add(s01[:], cs_flat[0:1, 0:NB], cs_flat[0:1, NB:2 * NB])
                s23 = misc.tile([1, NB], F32, tag="sm", bufs=3, name=f"s23_{nb}")
                nc.vector.tensor_add(s23[:], cs_flat[0:1, 2 * NB:3 * NB],
                                     cs_flat[0:1, 3 * NB:4 * NB])
                csum = misc.tile([1, NB], F32, tag="sm", bufs=3, name=f"csum_{nb}")
                nc.vector.tensor_add(csum[:], s01[:], s23[:])
                recip = misc.tile([1, NB], F32, tag="recip", bufs=2, name=f"recip_{nb}")
                nc.vector.reciprocal(recip[:], csum[:])
                pb_t = pcp.tile([128, NB], F32, tag="pc", name=f"pb_{nb}")
                nc.tensor.matmul(pb_t[:], onesr_sb[:], recip[:], start=True, stop=True)
                bc_sb = misc.tile([128, NB], F32, tag="bc", name=f"bc_{nb}")
                nc.vector.tensor_copy(bc_sb[:], pb_t[:])
                for ch in range(2):
                    tmp = misc.tile([128, NB], F32, tag="tmp", bufs=2,
                                    name=f"tmp_{nb}_{ch}")
                    nc.vector.tensor_mul(tmp[:], po_t[ch][:], bc_sb[:])
                    ot = misc.tile([128, NB], F32, tag="ot", bufs=2,
                                   name=f"ot_{nb}_{ch}")
                    nc.vector.scalar_tensor_tensor(
                        ot[:], tmp[:], gamma,
                        xt[(ch, nb // 2)][:, (nb % 2) * NB:(nb % 2 + 1) * NB],
                        ALU.mult, ALU.add)
                    nc.sync.dma_start(
                        out_d[ch * 128:(ch + 1) * 128, nb * NB:(nb + 1) * NB], ot[:])

    nc.compile()
    return nc


def kernel(**inputs) -> np.ndarray:
    x = np.asarray(inputs["x"], dtype=np.float32)
    wq = np.asarray(inputs["wq"], dtype=np.float32)
    bq = np.asarray(inputs["bq"], dtype=np.float32)
    wk = np.asarray(inputs["wk"], dtype=np.float32)
    bk = np.asarray(inputs["bk"], dtype=np.float32)
    wv = np.asarray(inputs["wv"], dtype=np.float32)
    bv = np.asarray(inputs["bv"], dtype=np.float32)
    gamma = float(np.asarray(inputs["gamma"]).reshape(-1)[0])

    scale = float(R) ** -0.5
    wqt = np.ascontiguousarray((wq * scale).T)           # (C, R)
    wkt = np.ascontiguousarray(wk.T)                     # (C, R)
    wvt = np.ascontiguousarray(wv.T)                     # (C, C)
    bq4 = np.ascontiguousarray(np.tile(bq * scale, 4).reshape(128, 1))
    bk4 = np.ascontiguousarray(np.tile(bk, 4).reshape(128, 1))
    bv_row = np.ascontiguousarray(bv.reshape(1, C))
    onesc = np.ones((128, 1), dtype=np.float32)
    onesr = np.ones((1, 128), dtype=np.float32)

    key = gamma
    if key not in _cache:
        _cache.clear()
        _cache[key] = _build_program(gamma)
    nc = _cache[key]

    shared = dict(wqt=wqt, wkt=wkt, wvt=wvt, bq4=bq4, bk4=bk4, bv=bv_row,
                  onesc=onesc, onesr=onesr)
    in_maps = []
    for b in range(B):
        m = dict(shared)
        m["x"] = np.ascontiguousarray(x[b].reshape(C, N))
        in_maps.append(m)

    res = bass_utils.run_bass_kernel_spmd(nc, in_maps, core_ids=list(range(NCORES)))
    out = np.stack([res.results[b]["out"].reshape(C, HH, WW) for b in range(B)])
    return out.astype(np.float32)


# revision 5
# speedup vs baseline: 186.1470x; 186.1470x over previous
"""Trainium2 Bass kernel for LocalWindowAttention (B=8, C=256, H=W=64, r=32).

Strategy: data-parallel over batch across 8 NeuronCores (one batch element
per core, zero collectives).  Per core everything is computed in a
"transposed" layout so that the softmax denominator (a sum over the
contraction axis m, which lives on SBUF partitions) can be computed with
cheap TensorE ones-matmuls instead of the slow VectorE:

  qT  (r=32 x N)   = (wq*scale) @ x + bq*scale   replicated to 4 row groups
  kT  (r=32 x N)   = wk @ x + bk                 replicated to 4 row groups
  vT  (m x C)      = x^T @ wv^T + bv             (ones-matmul bias trick)
  sT  (m x n)      = kT_slice^T @ qT             4x row-tiled (K=32 packing)
  E = exp(sT)      on ScalarE, PSUM -> SBUF
  colsum           = ones^T @ E   per m-chunk, 4x col-tiled into one bank
  out_u (C x n)    = vT^T @ E     accumulated over m-chunks
  out = x + gamma * out_u / colsum
"""

import numpy as np
from contextlib import ExitStack

import concourse.bass as bass
import concourse.tile as tile
from concourse import bacc, mybir, bass_utils

F32 = mybir.dt.float32
AF = mybir.ActivationFunctionType
ALU = mybir.AluOpType

B, C, HH, WW = 8, 256, 64, 64
N = HH * WW            # 4096 tokens
R = 32                 # low-rank q/k dim
NCORES = 8
NB = 512               # n-block (free dim per matmul)
NNB = N // NB          # 8
MC = 128               # m-chunk (contraction tile)
NMC = N // MC          # 32

_cache = {}


def _build_program(gamma: float):
    nc = bacc.Bacc("TRN2", debug=False, enable_asserts=True, num_devices=NCORES)
    x_d = nc.dram_tensor("x", (C, N), F32, kind="ExternalInput").ap()
    wqt_d = nc.dram_tensor("wqt", (C, R), F32, kind="ExternalInput").ap()
    wkt_d = nc.dram_tensor("wkt", (C, R), F32, kind="ExternalInput").ap()
    wvt_d = nc.dram_tensor("wvt", (C, C), F32, kind="ExternalInput").ap()
    bq4_d = nc.dram_tensor("bq4", (128, 1), F32, kind="ExternalInput").ap()
    bk4_d = nc.dram_tensor("bk4", (128, 1), F32, kind="ExternalInput").ap()
    bv_d = nc.dram_tensor("bv", (1, C), F32, kind="ExternalInput").ap()
    onesc_d = nc.dram_tensor("onesc", (128, 1), F32, kind="ExternalInput").ap()
    onesr_d = nc.dram_tensor("onesr", (1, 128), F32, kind="ExternalInput").ap()
    out_d = nc.dram_tensor("out", (C, N), F32, kind="ExternalOutput").ap()

    with tile.TileContext(nc) as tc, ExitStack() as ctx:
        consts = ctx.enter_context(tc.tile_pool(name="consts", bufs=1))
        xpool = ctx.enter_context(tc.tile_pool(name="xp", bufs=8))
        qkpool = ctx.enter_context(tc.tile_pool(name="qk", bufs=1))
        vpool = ctx.enter_context(tc.tile_pool(name="vp", bufs=1))
        exppool = ctx.enter_context(tc.tile_pool(name="ep", bufs=1))
        misc = ctx.enter_context(tc.tile_pool(name="misc", bufs=1))

        # ---- constants ----
        wqt_sb = consts.tile([128, 2, R], F32, tag="wqt")
        nc.sync.dma_start(wqt_sb[:], wqt_d.rearrange("(k p) r -> p k r", p=128))
        wkt_sb = consts.tile([128, 2, R], F32, tag="wkt")
        nc.sync.dma_start(wkt_sb[:], wkt_d.rearrange("(k p) r -> p k r", p=128))
        wvt_sb = consts.tile([128, 2, C], F32, tag="wvt")
        nc.sync.dma_start(wvt_sb[:], wvt_d.rearrange("(k p) c -> p k c", p=128))
        bq4_sb = consts.tile([128, 1], F32, tag="bq4")
        nc.sync.dma_start(bq4_sb[:], bq4_d[:])
        bk4_sb = consts.tile([128, 1], F32, tag="bk4")
        nc.sync.dma_start(bk4_sb[:], bk4_d[:])
        bv_sb = consts.tile([1, C], F32, tag="bv")
        nc.sync.dma_start(bv_sb[:], bv_d[:])
        onesc_sb = consts.tile([128, 1], F32, tag="onesc")
        nc.sync.dma_start(onesc_sb[:], onesc_d[:])
        onesr_sb = consts.tile([1, 128], F32, tag="onesr")
        nc.sync.dma_start(onesr_sb[:], onesr_d[:])

        # ---- persistent activations ----
        qt_sb = qkpool.tile([128, N], F32, tag="qt")     # q^T replicated 4x
        kt_sb = qkpool.tile([128, N], F32, tag="kt")     # k^T replicated 4x
        vt_sb = vpool.tile([128, NMC * C], F32, tag="vt")  # v^T, chunk j at [:, j*C:(j+1)*C]
        exp_sb = exppool.tile([128, NMC * NB], F32, tag="exp")  # one n-block of exp(sT)

        xt = {}

        # ================= phase 1: q/k/v projections =================
        with tc.tile_pool(name="pp", bufs=2, space=bass.MemorySpace.PSUM) as pp, \
             tc.tile_pool(name="pvp", bufs=2, space=bass.MemorySpace.PSUM) as pvp:
            for qq in range(4):
                for k in range(2):
                    t_ = xpool.tile([128, 1024], F32, tag="xt")
                    nc.sync.dma_start(
                        t_[:], x_d[k * 128:(k + 1) * 128, qq * 1024:(qq + 1) * 1024])
                    xt[(k, qq)] = t_

                # q and k projections for the two 512-col chunks of this quarter,
                # replicated into 4 col groups (-> 4 partition groups of 32)
                for half in range(2):
                    i = 2 * qq + half
                    for (w_sb, b_sb, dst) in ((wqt_sb, bq4_sb, qt_sb),
                                              (wkt_sb, bk4_sb, kt_sb)):
                        pt = pp.tile([128, NB], F32, tag="pp")
                        for g in range(4):
                            for k in range(2):
                                nc.tensor.matmul(
                                    pt[32 * g:32 * (g + 1), :],
                                    w_sb[:, k, :],
                                    xt[(k, qq)][:, half * NB:(half + 1) * NB],
                                    start=(k == 0), stop=(k == 1),
                                    tile_position=(0, 32 * g))
                        nc.vector.tensor_scalar_add(
                            dst[:, i * NB:(i + 1) * NB], pt[:], b_sb[:])

                # vT for the 8 m-chunks of this quarter
                for jj in range(8):
                    j = 8 * qq + jj
                    pv = pvp.tile([128, C], F32, tag="pv")
                    nc.tensor.matmul(pv[:], xt[(0, qq)][:, jj * 128:(jj + 1) * 128],
                                     wvt_sb[:, 0, :], start=True, stop=False)
                    nc.tensor.matmul(pv[:], xt[(1, qq)][:, jj * 128:(jj + 1) * 128],
                                     wvt_sb[:, 1, :], start=False, stop=False)
                    nc.tensor.matmul(pv[:], onesr_sb[:], bv_sb[:],
                                     start=False, stop=True)
                    nc.vector.tensor_copy(vt_sb[:, j * C:(j + 1) * C], pv[:])

        # ================= phase 2: attention =================
        with tc.tile_pool(name="ps", bufs=2, space=bass.MemorySpace.PSUM) as psp, \
             tc.tile_pool(name="po", bufs=2, space=bass.MemorySpace.PSUM) as pop, \
             tc.tile_pool(name="pc", bufs=2, space=bass.MemorySpace.PSUM) as pcp:
            for nb in range(NNB):
                po_t = [pop.tile([128, NB], F32, tag="po", name=f"po_{nb}_{i}")
                        for i in range(2)]
                pc_t = pcp.tile([128, NB], F32, tag="pc")

                def consume(tt, po_t=po_t, pc_t=pc_t, nb=nb):
                    # out_u matmuls for the 2 chunks of exp group tt
                    for g2 in range(2):
                        j = 2 * tt + g2
                        for ch in range(2):
                            nc.tensor.matmul(
                                po_t[ch][:],
                                vt_sb[:, j * C + ch * 128: j * C + (ch + 1) * 128],
                                exp_sb[:, j * NB:(j + 1) * NB],
                                start=(j == 0), stop=(j == NMC - 1))
                    # colsum matmuls: 4 chunks at a time, 4x col-tiled
                    if tt % 2 == 1:
                        t2 = tt // 2
                        for g in range(4):
                            j = 4 * t2 + g
                            nc.tensor.matmul(
                                pc_t[32 * g:32 * g + 1, :],
                                onesc_sb[:],
                                exp_sb[:, j * NB:(j + 1) * NB],
                                start=(t2 == 0), stop=(t2 == 7),
                                tile_position=(0, 32 * g))

                for t in range(16):
                    ps_t = psp.tile([128, 2 * NB], F32, tag="ps")
                    for g2 in range(2):
                        j = 2 * t + g2
                        gm = j % 4
                        nc.tensor.matmul(
                            ps_t[:, g2 * NB:(g2 + 1) * NB],
                            kt_sb[32 * gm:32 * (gm + 1), j * 128:(j + 1) * 128],
                            qt_sb[32 * gm:32 * (gm + 1), nb * NB:(nb + 1) * NB],
                            start=True, stop=True,
                            tile_position=(32 * gm, 0))
                    nc.scalar.activation(
                        exp_sb[:, 2 * t * NB:(2 * t + 2) * NB], ps_t[:], AF.Exp)
                    if t >= 1:
                        consume(t - 1)
                consume(15)

                # ---- softmax denominator combine + normalize ----
                cs_stage = misc.tile([128, NB], F32, tag="cs", name=f"cs_{nb}")
                nc.vector.tensor_copy(cs_stage[0:97, :], pc_t[0:97, :])
                cs_flat = misc.tile([1, 4 * NB], F32, tag="csf", name=f"csf_{nb}")
                for g in range(4):
                    nc.sync.dma_start(cs_flat[0:1, g * NB:(g + 1) * NB],
                                      cs_stage[32 * g:32 * g + 1, :])
                s01 = misc.tile([1, NB], F32, tag="sm", bufs=3, name=f"s01_{nb}")
                nc.vector.tensor_add(s01[:], cs_flat[0:1, 0:NB], cs_flat[0:1, NB:2 * NB])
                s23 = misc.tile([1, NB], F32, tag="sm", bufs=3, name=f"s23_{nb}")
                nc.vector.tensor_add(s23[:], cs_flat[0:1, 2 * NB:3 * NB],
                                     cs_flat[0:1, 3 * NB:4 * NB])
                csum = misc.tile([1, NB], F32, tag="sm", bufs=3, name=f"csum_{nb}")
                nc.vector.tensor_add(csum[:], s01[:], s23[:])
                recip = misc.tile([1, NB], F32, tag="recip", bufs=2, name=f"recip_{nb}")
                nc.vector.reciprocal(recip[:], csum[:])
                pb_t = pcp.tile([128, NB], F32, tag="pc", name=f"pb_{nb}")
                nc.tensor.matmul(pb_t[:], onesr_sb[:], recip[:], start=True, stop=True)
                bc_sb = misc.tile([128, NB], F32, tag="bc", name=f"bc_{nb}")
                nc.vector.tensor_copy(bc_sb[:], pb_t[:])
                for ch in range(2):
                    tmp = misc.tile([128, NB], F32, tag="tmp", bufs=2,
                                    name=f"tmp_{nb}_{ch}")
                    nc.vector.tensor_mul(tmp[:], po_t[ch][:], bc_sb[:])
                    ot = misc.tile([128, NB], F32, tag="ot", bufs=2,
                                   name=f"ot_{nb}_{ch}")
                    nc.vector.scalar_tensor_tensor(
                        ot[:], tmp[:], gamma,
                        xt[(ch, nb // 2)][:, (nb % 2) * NB:(nb % 2 + 1) * NB],
                        ALU.mult, ALU.add)
                    nc.sync.dma_start(
                        out_d[ch * 128:(ch + 1) * 128, nb * NB:(nb + 1) * NB], ot[:])

    nc.compile()
    return nc


def _make_in_maps(inputs):
    x = np.asarray(inputs["x"], dtype=np.float32)
    wq = np.asarray(inputs["wq"], dtype=np.float32)
    bq = np.asarray(inputs["bq"], dtype=np.float32)
    wk = np.asarray(inputs["wk"], dtype=np.float32)
    bk = np.asarray(inputs["bk"], dtype=np.float32)
    wv = np.asarray(inputs["wv"], dtype=np.float32)
    bv = np.asarray(inputs["bv"], dtype=np.float32)

    scale = float(R) ** -0.5
    shared = dict(
        wqt=np.ascontiguousarray((wq * scale).T),        # (C, R)
        wkt=np.ascontiguousarray(wk.T),                  # (C, R)
        wvt=np.ascontiguousarray(wv.T),                  # (C, C)
        bq4=np.ascontiguousarray(np.tile(bq * scale, 4).reshape(128, 1)),
        bk4=np.ascontiguousarray(np.tile(bk, 4).reshape(128, 1)),
        bv=np.ascontiguousarray(bv.reshape(1, C)),
        onesc=np.ones((128, 1), dtype=np.float32),
        onesr=np.ones((1, 128), dtype=np.float32),
    )
    in_maps = []
    for b in range(B):
        m = dict(shared)
        m["x"] = np.ascontiguousarray(x[b].reshape(C, N))
        in_maps.append(m)
    return in_maps


def kernel(**inputs) -> np.ndarray:
    gamma = float(np.asarray(inputs["gamma"]).reshape(-1)[0])
    key = gamma
    if key not in _cache:
        _cache.clear()
        _cache[key] = _build_program(gamma)
    nc = _cache[key]

    in_maps = _make_in_maps(inputs)
    res = bass_utils.run_bass_kernel_spmd(nc, in_maps, core_ids=list(range(NCORES)))
    out = np.stack([res.results[b]["out"].reshape(C, HH, WW) for b in range(B)])
    return out.astype(np.float32)


# revision 11
# speedup vs baseline: 253.9866x; 1.3644x over previous
"""Trainium2 Bass kernel for LocalWindowAttention (B=8, C=256, H=W=64, r=32).

Strategy: data-parallel over batch across 8 NeuronCores (one batch element
per core, zero collectives).  Per core everything is computed in a
"transposed" layout so that the softmax denominator (a sum over the
contraction axis m, which lives on SBUF partitions) can be computed with
cheap TensorE ones-matmuls instead of the slow VectorE:

  qT  (r=32 x N)   = (wq*scale) @ x + bq*scale   replicated to 4 row groups
  kT  (r=32 x N)   = wk @ x + bk                 replicated to 4 row groups
  vT  (m x C)      = x^T @ wv^T + bv             (ones-matmul bias trick)
  sT  (m x n)      = kT_slice^T @ qT             4x row-tiled (K=32 packing)
  E = exp(sT)      on ScalarE, PSUM -> SBUF
  colsum           = ones^T @ E   per m-chunk, 4x col-tiled into one bank
  out_u (C x n)    = vT^T @ E     accumulated over m-chunks
  out = x + gamma * out_u / colsum
"""

import numpy as np
from contextlib import ExitStack

import concourse.bass as bass
import concourse.tile as tile
from concourse import bacc, mybir, bass_utils

F32 = mybir.dt.float32
AF = mybir.ActivationFunctionType
ALU = mybir.AluOpType
F32R = mybir.dt.float32r


B, C, HH, WW = 8, 256, 64, 64
N = HH * WW            # 4096 tokens
R = 32                 # low-rank q/k dim
NCORES = 8
NB = 512               # n-block (free dim per matmul)
NNB = N // NB          # 8
MC = 128               # m-chunk (contraction tile)
NMC = N // MC          # 32

_cache = {}


def _build_program(gamma: float):
    nc = bacc.Bacc("TRN2", debug=False, enable_asserts=True, num_devices=NCORES)
    x_d = nc.dram_tensor("x", (C, N), F32R, kind="ExternalInput").ap()
    wqt_d = nc.dram_tensor("wqt", (C, R), F32R, kind="ExternalInput").ap()
    wkt_d = nc.dram_tensor("wkt", (C, R), F32R, kind="ExternalInput").ap()
    wvt_d = nc.dram_tensor("wvt", (C, C), F32R, kind="ExternalInput").ap()
    bq4_d = nc.dram_tensor("bq4", (128, 1), F32, kind="ExternalInput").ap()
    bk4_d = nc.dram_tensor("bk4", (128, 1), F32, kind="ExternalInput").ap()
    bv_d = nc.dram_tensor("bv", (1, C), F32R, kind="ExternalInput").ap()
    onesc_d = nc.dram_tensor("onesc", (128, 1), F32R, kind="ExternalInput").ap()
    onesr_d = nc.dram_tensor("onesr", (1, 128), F32R, kind="ExternalInput").ap()
    out_d = nc.dram_tensor("out", (C, N), F32, kind="ExternalOutput").ap()

    with tile.TileContext(nc) as tc, ExitStack() as ctx, \
         nc.allow_low_precision(reason="fp32r matmul streaming mode"):
        consts = ctx.enter_context(tc.tile_pool(name="consts", bufs=1))
        xpool = ctx.enter_context(tc.tile_pool(name="xp", bufs=8))
        qkpool = ctx.enter_context(tc.tile_pool(name="qk", bufs=1))
        vpool = ctx.enter_context(tc.tile_pool(name="vp", bufs=1))
        exppool = ctx.enter_context(tc.tile_pool(name="ep", bufs=1))
        misc = ctx.enter_context(tc.tile_pool(name="misc", bufs=1))

        # ---- constants ----
        wqt_sb = consts.tile([128, 2, R], F32R, tag="wqt")
        nc.sync.dma_start(wqt_sb[:], wqt_d.rearrange("(k p) r -> p k r", p=128))
        wkt_sb = consts.tile([128, 2, R], F32R, tag="wkt")
        nc.sync.dma_start(wkt_sb[:], wkt_d.rearrange("(k p) r -> p k r", p=128))
        wvt_sb = consts.tile([128, 2, C], F32R, tag="wvt")
        nc.sync.dma_start(wvt_sb[:], wvt_d.rearrange("(k p) c -> p k c", p=128))
        bq4_sb = consts.tile([128, 1], F32, tag="bq4")
        nc.sync.dma_start(bq4_sb[:], bq4_d[:])
        bk4_sb = consts.tile([128, 1], F32, tag="bk4")
        nc.sync.dma_start(bk4_sb[:], bk4_d[:])
        bv_sb = consts.tile([1, C], F32R, tag="bv")
        nc.sync.dma_start(bv_sb[:], bv_d[:])
        onesc_sb = consts.tile([128, 1], F32R, tag="onesc")
        nc.sync.dma_start(onesc_sb[:], onesc_d[:])
        onesr_sb = consts.tile([1, 128], F32R, tag="onesr")
        nc.sync.dma_start(onesr_sb[:], onesr_d[:])

        # ---- persistent activations ----
        qt_sb = qkpool.tile([128, N], F32R, tag="qt")     # q^T replicated 4x
        kt_sb = qkpool.tile([128, N], F32R, tag="kt")     # k^T replicated 4x
        vt_sb = vpool.tile([128, NMC * C], F32R, tag="vt")  # v^T, chunk j at [:, j*C:(j+1)*C]
        exp_sb = exppool.tile([128, NMC * NB], F32R, tag="exp")  # one n-block of exp(sT)

        xt = {}

        # ================= phase 1: q/k/v projections =================
        with tc.tile_pool(name="pp", bufs=2, space=bass.MemorySpace.PSUM) as pp, \
             tc.tile_pool(name="pvp", bufs=2, space=bass.MemorySpace.PSUM) as pvp:
            for qq in range(4):
                for k in range(2):
                    t_ = xpool.tile([128, 1024], F32R, tag="xt")
                    nc.sync.dma_start(
                        t_[:], x_d[k * 128:(k + 1) * 128, qq * 1024:(qq + 1) * 1024])
                    xt[(k, qq)] = t_

                # q and k projections for the two 512-col chunks of this quarter,
                # replicated into 4 col groups (-> 4 partition groups of 32)
                for half in range(2):
                    i = 2 * qq + half
                    for (w_sb, b_sb, dst) in ((wqt_sb, bq4_sb, qt_sb),
                                              (wkt_sb, bk4_sb, kt_sb)):
                        pt = pp.tile([128, NB], F32, tag="pp")
                        for k in range(2):
                            nc.tensor.matmul(
                                pt[0:32, :],
                                w_sb[:, k, :],
                                xt[(k, qq)][:, half * NB:(half + 1) * NB],
                                start=(k == 0), stop=(k == 1))
                        nc.vector.tensor_scalar_add(
                            dst[0:32, i * NB:(i + 1) * NB], pt[0:32, :],
                            b_sb[0:32, :])
                # replicate rows 0:32 -> 32:64, 64:96, 96:128 for row tiling
                for dst in (qt_sb, kt_sb):
                    for g in range(1, 4):
                        nc.sync.dma_start(
                            dst[32 * g:32 * (g + 1), qq * 1024:(qq + 1) * 1024],
                            dst[0:32, qq * 1024:(qq + 1) * 1024])

                # vT for the 8 m-chunks of this quarter
                for jj in range(8):
                    j = 8 * qq + jj
                    pv = pvp.tile([128, C], F32, tag="pv")
                    nc.tensor.matmul(pv[:], xt[(0, qq)][:, jj * 128:(jj + 1) * 128],
                                     wvt_sb[:, 0, :], start=True, stop=False)
                    nc.tensor.matmul(pv[:], xt[(1, qq)][:, jj * 128:(jj + 1) * 128],
                                     wvt_sb[:, 1, :], start=False, stop=False)
                    nc.tensor.matmul(pv[:], onesr_sb[:], bv_sb[:],
                                     start=False, stop=True)
                    nc.vector.tensor_copy(vt_sb[:, j * C:(j + 1) * C], pv[:])

        # ================= phase 2: attention =================
        with tc.tile_pool(name="ps", bufs=2, space=bass.MemorySpace.PSUM) as psp, \
             tc.tile_pool(name="po", bufs=2, space=bass.MemorySpace.PSUM) as pop, \
             tc.tile_pool(name="pc", bufs=2, space=bass.MemorySpace.PSUM) as pcp:
            for nb in range(NNB):
                po_t = [pop.tile([128, NB], F32, tag="po", name=f"po_{nb}_{i}")
                        for i in range(2)]
                pc_t = pcp.tile([128, NB], F32, tag="pc")

                def consume(tt, po_t=po_t, pc_t=pc_t, nb=nb):
                    # out_u matmuls for the 2 chunks of exp group tt
                    for g2 in range(2):
                        j = 2 * tt + g2
                        for ch in range(2):
                            nc.tensor.matmul(
                                po_t[ch][:],
                                vt_sb[:, j * C + ch * 128: j * C + (ch + 1) * 128],
                                exp_sb[:, j * NB:(j + 1) * NB],
                                start=(j == 0), stop=(j == NMC - 1))
                    # colsum matmuls (fp32r, M=1, full rate)
                    for g2 in range(2):
                        j = 2 * tt + g2
                        nc.tensor.matmul(
                            pc_t[0:1, :],
                            onesc_sb[:],
                            exp_sb[:, j * NB:(j + 1) * NB],
                            start=(j == 0), stop=(j == NMC - 1))

                for t in range(16):
                    ps_t = psp.tile([128, 2 * NB], F32, tag="ps")
                    for g2 in range(2):
                        j = 2 * t + g2
                        gm = j % 4
                        nc.tensor.matmul(
                            ps_t[:, g2 * NB:(g2 + 1) * NB],
                            kt_sb[32 * gm:32 * (gm + 1), j * 128:(j + 1) * 128],
                            qt_sb[32 * gm:32 * (gm + 1), nb * NB:(nb + 1) * NB],
                            start=True, stop=True,
                            tile_position=(32 * gm, 0))
                    nc.scalar.activation(
                        exp_sb[:, 2 * t * NB:(2 * t + 2) * NB], ps_t[:], AF.Exp)
                    if t >= 1:
                        consume(t - 1)
                consume(15)

                # ---- softmax denominator -> reciprocal ----
                recip = misc.tile([1, NB], F32R, tag="recip", bufs=2, name=f"recip_{nb}")
                nc.vector.reciprocal(recip[:], pc_t[0:1, :])
                pb_t = pcp.tile([128, NB], F32, tag="pc", name=f"pb_{nb}")
                nc.tensor.matmul(pb_t[:], onesr_sb[:], recip[:], start=True, stop=True)
                bc_sb = misc.tile([128, NB], F32, tag="bc", name=f"bc_{nb}")
                nc.vector.tensor_copy(bc_sb[:], pb_t[:])
                for ch in range(2):
                    tmp = misc.tile([128, NB], F32, tag="tmp", bufs=2,
                                    name=f"tmp_{nb}_{ch}")
                    nc.vector.tensor_mul(tmp[:], po_t[ch][:], bc_sb[:])
                    ot = misc.tile([128, NB], F32, tag="ot", bufs=2,
                                   name=f"ot_{nb}_{ch}")
                    nc.vector.scalar_tensor_tensor(
                        ot[:], tmp[:], gamma,
                        xt[(ch, nb // 2)][:, (nb % 2) * NB:(nb % 2 + 1) * NB].bitcast(F32),
                        ALU.mult, ALU.add)
                    nc.sync.dma_start(
                        out_d[ch * 128:(ch + 1) * 128, nb * NB:(nb + 1) * NB], ot[:])

    nc.compile()
    return nc


def _make_in_maps(inputs):
    x = np.asarray(inputs["x"], dtype=np.float32)
    wq = np.asarray(inputs["wq"], dtype=np.float32)
    bq = np.asarray(inputs["bq"], dtype=np.float32)
    wk = np.asarray(inputs["wk"], dtype=np.float32)
    bk = np.asarray(inputs["bk"], dtype=np.float32)
    wv = np.asarray(inputs["wv"], dtype=np.float32)
    bv = np.asarray(inputs["bv"], dtype=np.float32)

    scale = float(R) ** -0.5
    shared = dict(
        wqt=np.ascontiguousarray((wq * scale).T),        # (C, R)
        wkt=np.ascontiguousarray(wk.T),                  # (C, R)
        wvt=np.ascontiguousarray(wv.T),                  # (C, C)
        bq4=np.ascontiguousarray(np.tile(bq * scale, 4).reshape(128, 1)),
        bk4=np.ascontiguousarray(np.tile(bk, 4).reshape(128, 1)),
        bv=np.ascontiguousarray(bv.reshape(1, C)),
        onesc=np.ones((128, 1), dtype=np.float32),
        onesr=np.ones((1, 128), dtype=np.float32),
    )
    in_maps = []
    for b in range(B):
        m = dict(shared)
        m["x"] = np.ascontiguousarray(x[b].reshape(C, N))
        in_maps.append(m)
    return in_maps


def kernel(**inputs) -> np.ndarray:
    gamma = float(np.asarray(inputs["gamma"]).reshape(-1)[0])
    key = gamma
    if key not in _cache:
        _cache.clear()
        _cache[key] = _build_program(gamma)
    nc = _cache[key]

    in_maps = _make_in_maps(inputs)
    res = bass_utils.run_bass_kernel_spmd(nc, in_maps, core_ids=list(range(NCORES)))
    out = np.stack([res.results[b]["out"].reshape(C, HH, WW) for b in range(B)])
    return out.astype(np.float32)


# revision 14
# speedup vs baseline: 261.2776x; 1.0287x over previous
"""Trainium2 Bass kernel for LocalWindowAttention (B=8, C=256, H=W=64, r=32).

Strategy: data-parallel over batch across 8 NeuronCores (one batch element
per core, zero collectives).  Per core everything is computed in a
"transposed" layout so that the softmax denominator (a sum over the
contraction axis m, which lives on SBUF partitions) can be computed with
cheap TensorE ones-matmuls instead of the slow VectorE:

  qT  (r=32 x N)   = (wq*scale) @ x + bq*scale   replicated to 4 row groups
  kT  (r=32 x N)   = wk @ x + bk                 replicated to 4 row groups
  vT  (m x C)      = x^T @ wv^T + bv             (ones-matmul bias trick)
  sT  (m x n)      = kT_slice^T @ qT             4x row-tiled (K=32 packing)
  E = exp(sT)      on ScalarE, PSUM -> SBUF
  colsum           = ones^T @ E   per m-chunk, 4x col-tiled into one bank
  out_u (C x n)    = vT^T @ E     accumulated over m-chunks
  out = x + gamma * out_u / colsum
"""

import numpy as np
from contextlib import ExitStack

import concourse.bass as bass
import concourse.tile as tile
from concourse import bacc, mybir, bass_utils

F32 = mybir.dt.float32
AF = mybir.ActivationFunctionType
ALU = mybir.AluOpType
F32R = mybir.dt.float32r


B, C, HH, WW = 8, 256, 64, 64
N = HH * WW            # 4096 tokens
R = 32                 # low-rank q/k dim
NCORES = 8
NB = 512               # n-block (free dim per matmul)
NNB = N // NB          # 8
MC = 128               # m-chunk (contraction tile)
NMC = N // MC          # 32

_cache = {}


def _build_program(gamma: float):
    nc = bacc.Bacc("TRN2", debug=False, enable_asserts=True, num_devices=NCORES)
    x_d = nc.dram_tensor("x", (C, N), F32R, kind="ExternalInput").ap()
    wqt_d = nc.dram_tensor("wqt", (C, R), F32R, kind="ExternalInput").ap()
    wkt_d = nc.dram_tensor("wkt", (C, R), F32R, kind="ExternalInput").ap()
    wvt_d = nc.dram_tensor("wvt", (C, C), F32R, kind="ExternalInput").ap()
    bq4_d = nc.dram_tensor("bq4", (128, 1), F32, kind="ExternalInput").ap()
    bk4_d = nc.dram_tensor("bk4", (128, 1), F32, kind="ExternalInput").ap()
    bv_d = nc.dram_tensor("bv", (1, C), F32R, kind="ExternalInput").ap()
    onesc_d = nc.dram_tensor("onesc", (128, 1), F32R, kind="ExternalInput").ap()
    onesr_d = nc.dram_tensor("onesr", (1, 128), F32R, kind="ExternalInput").ap()
    out_d = nc.dram_tensor("out", (C, N), F32, kind="ExternalOutput").ap()

    with tile.TileContext(nc) as tc, ExitStack() as ctx, \
         nc.allow_low_precision(reason="fp32r matmul streaming mode"):
        consts = ctx.enter_context(tc.tile_pool(name="consts", bufs=1))
        xpool = ctx.enter_context(tc.tile_pool(name="xp", bufs=8))
        qkpool = ctx.enter_context(tc.tile_pool(name="qk", bufs=1))
        vpool = ctx.enter_context(tc.tile_pool(name="vp", bufs=1))
        exppool = ctx.enter_context(tc.tile_pool(name="ep", bufs=1))
        misc = ctx.enter_context(tc.tile_pool(name="misc", bufs=1))

        # ---- constants ----
        wqt_sb = consts.tile([128, 2, R], F32R, tag="wqt")
        nc.sync.dma_start(wqt_sb[:], wqt_d.rearrange("(k p) r -> p k r", p=128))
        wkt_sb = consts.tile([128, 2, R], F32R, tag="wkt")
        nc.sync.dma_start(wkt_sb[:], wkt_d.rearrange("(k p) r -> p k r", p=128))
        wvt_sb = consts.tile([128, 2, C], F32R, tag="wvt")
        bq4_sb = consts.tile([128, 1], F32, tag="bq4")
        nc.sync.dma_start(bq4_sb[:], bq4_d[:])
        bk4_sb = consts.tile([128, 1], F32, tag="bk4")
        nc.sync.dma_start(bk4_sb[:], bk4_d[:])
        bv_sb = consts.tile([1, C], F32R, tag="bv")
        onesc_sb = consts.tile([128, 1], F32R, tag="onesc")
        nc.sync.dma_start(onesc_sb[:], onesc_d[:])
        onesr_sb = consts.tile([1, 128], F32R, tag="onesr")
        nc.sync.dma_start(onesr_sb[:], onesr_d[:])

        # ---- persistent activations ----
        qt_sb = qkpool.tile([128, N], F32R, tag="qt")     # q^T replicated 4x
        kt_sb = qkpool.tile([128, N], F32R, tag="kt")     # k^T replicated 4x
        vt_sb = vpool.tile([128, NMC * C], F32R, tag="vt")  # v^T, chunk j at [:, j*C:(j+1)*C]
        exp_sb = exppool.tile([128, NMC * NB], F32R, tag="exp")  # one n-block of exp(sT)

        xt = {}
        _deferred_const_dmas = [
            lambda: nc.sync.dma_start(
                wvt_sb[:], wvt_d.rearrange("(k p) c -> p k c", p=128)),
            lambda: nc.sync.dma_start(bv_sb[:], bv_d[:]),
        ]

        # ================= phase 1: q/k/v projections =================
        with tc.tile_pool(name="pp", bufs=2, space=bass.MemorySpace.PSUM) as pp, \
             tc.tile_pool(name="pvp", bufs=2, space=bass.MemorySpace.PSUM) as pvp:
            for qq in range(4):
                for k in range(2):
                    t_ = xpool.tile([128, 1024], F32R, tag="xt")
                    for hf in range(2):
                        nc.sync.dma_start(
                            t_[:, hf * 512:(hf + 1) * 512],
                            x_d[k * 128:(k + 1) * 128,
                                qq * 1024 + hf * 512:qq * 1024 + (hf + 1) * 512])
                    xt[(k, qq)] = t_
                if qq == 0:
                    for fire in _deferred_const_dmas:
                        fire()
                    _deferred_const_dmas = []

                # q and k projections for the two 512-col chunks of this quarter,
                # replicated into 4 col groups (-> 4 partition groups of 32)
                for half in range(2):
                    i = 2 * qq + half
                    for (w_sb, b_sb, dst) in ((wqt_sb, bq4_sb, qt_sb),
                                              (wkt_sb, bk4_sb, kt_sb)):
                        pt = pp.tile([128, NB], F32, tag="pp")
                        for k in range(2):
                            nc.tensor.matmul(
                                pt[0:32, :],
                                w_sb[:, k, :],
                                xt[(k, qq)][:, half * NB:(half + 1) * NB],
                                start=(k == 0), stop=(k == 1))
                        nc.vector.tensor_scalar_add(
                            dst[0:32, i * NB:(i + 1) * NB], pt[0:32, :],
                            b_sb[0:32, :])
                # replicate rows 0:32 -> 32:64, 64:96, 96:128 for row tiling
                for dst in (qt_sb, kt_sb):
                    for g in range(1, 4):
                        nc.sync.dma_start(
                            dst[32 * g:32 * (g + 1), qq * 1024:(qq + 1) * 1024],
                            dst[0:32, qq * 1024:(qq + 1) * 1024])

                # vT for the 8 m-chunks of this quarter
                for jj in range(8):
                    j = 8 * qq + jj
                    pv = pvp.tile([128, C], F32, tag="pv")
                    nc.tensor.matmul(pv[:], xt[(0, qq)][:, jj * 128:(jj + 1) * 128],
                                     wvt_sb[:, 0, :], start=True, stop=False)
                    nc.tensor.matmul(pv[:], xt[(1, qq)][:, jj * 128:(jj + 1) * 128],
                                     wvt_sb[:, 1, :], start=False, stop=False)
                    nc.tensor.matmul(pv[:], onesr_sb[:], bv_sb[:],
                                     start=False, stop=True)
                    nc.vector.tensor_copy(vt_sb[:, j * C:(j + 1) * C], pv[:])

        # ================= phase 2: attention =================
        with tc.tile_pool(name="ps", bufs=2, space=bass.MemorySpace.PSUM) as psp, \
             tc.tile_pool(name="po", bufs=2, space=bass.MemorySpace.PSUM) as pop, \
             tc.tile_pool(name="pc", bufs=2, space=bass.MemorySpace.PSUM) as pcp:
            for nb in range(NNB):
                po_t = [pop.tile([128, NB], F32, tag="po", name=f"po_{nb}_{i}")
                        for i in range(2)]
                pc_t = pcp.tile([128, NB], F32, tag="pc")

                def consume(tt, po_t=po_t, pc_t=pc_t, nb=nb):
                    # colsum matmuls (fp32r, M=1, full rate) - emitted first so
                    # the softmax-denominator tail clears as early as possible
                    for g2 in range(2):
                        j = 2 * tt + g2
                        nc.tensor.matmul(
                            pc_t[0:1, :],
                            onesc_sb[:],
                            exp_sb[:, j * NB:(j + 1) * NB],
                            start=(j == 0), stop=(j == NMC - 1))
                    # out_u matmuls for the 2 chunks of exp group tt
                    for g2 in range(2):
                        j = 2 * tt + g2
                        for ch in range(2):
                            nc.tensor.matmul(
                                po_t[ch][:],
                                vt_sb[:, j * C + ch * 128: j * C + (ch + 1) * 128],
                                exp_sb[:, j * NB:(j + 1) * NB],
                                start=(j == 0), stop=(j == NMC - 1))

                for t in range(16):
                    ps_t = psp.tile([128, 2 * NB], F32, tag="ps")
                    for g2 in range(2):
                        j = 2 * t + g2
                        gm = j % 4
                        nc.tensor.matmul(
                            ps_t[:, g2 * NB:(g2 + 1) * NB],
                            kt_sb[32 * gm:32 * (gm + 1), j * 128:(j + 1) * 128],
                            qt_sb[32 * gm:32 * (gm + 1), nb * NB:(nb + 1) * NB],
                            start=True, stop=True,
                            tile_position=(32 * gm, 0))
                    nc.scalar.activation(
                        exp_sb[:, 2 * t * NB:(2 * t + 2) * NB], ps_t[:], AF.Exp)
                    if t >= 1:
                        consume(t - 1)
                consume(15)

                # ---- softmax denominator -> reciprocal ----
                recip = misc.tile([1, NB], F32R, tag="recip", bufs=2, name=f"recip_{nb}")
                nc.vector.reciprocal(recip[:], pc_t[0:1, :])
                pb_t = pcp.tile([128, NB], F32, tag="pc", name=f"pb_{nb}")
                nc.tensor.matmul(pb_t[:], onesr_sb[:], recip[:], start=True, stop=True)
                bc_sb = misc.tile([128, NB], F32, tag="bc", name=f"bc_{nb}")
                nc.vector.tensor_copy(bc_sb[:], pb_t[:])
                for ch in range(2):
                    tmp = misc.tile([128, NB], F32, tag="tmp", bufs=2,
                                    name=f"tmp_{nb}_{ch}")
                    nc.vector.tensor_mul(tmp[:], po_t[ch][:], bc_sb[:])
                    ot = misc.tile([128, NB], F32, tag="ot", bufs=2,
                                   name=f"ot_{nb}_{ch}")
                    nc.vector.scalar_tensor_tensor(
                        ot[:], tmp[:], gamma,
                        xt[(ch, nb // 2)][:, (nb % 2) * NB:(nb % 2 + 1) * NB].bitcast(F32),
                        ALU.mult, ALU.add)
                    nc.sync.dma_start(
                        out_d[ch * 128:(ch + 1) * 128, nb * NB:(nb + 1) * NB], ot[:])

    nc.compile()
    return nc


def _make_in_maps(inputs):
    x = np.asarray(inputs["x"], dtype=np.float32)
    wq = np.asarray(inputs["wq"], dtype=np.float32)
    bq = np.asarray(inputs["bq"], dtype=np.float32)
    wk = np.asarray(inputs["wk"], dtype=np.float32)
    bk = np.asarray(inputs["bk"], dtype=np.float32)
    wv = np.asarray(inputs["wv"], dtype=np.float32)
    bv = np.asarray(inputs["bv"], dtype=np.float32)

    scale = float(R) ** -0.5
    shared = dict(
        wqt=np.ascontiguousarray((wq * scale).T),        # (C, R)
        wkt=np.ascontiguousarray(wk.T),                  # (C, R)
        wvt=np.ascontiguousarray(wv.T),                  # (C, C)
        bq4=np.ascontiguousarray(np.tile(bq * scale, 4).reshape(128, 1)),
        bk4=np.ascontiguousarray(np.tile(bk, 4).reshape(128, 1)),
        bv=np.ascontiguousarray(bv.reshape(1, C)),
        onesc=np.ones((128, 1), dtype=np.float32),
        onesr=np.ones((1, 128), dtype=np.float32),
    )
    in_maps = []
    for b in range(B):
        m = dict(shared)
        m["x"] = np.ascontiguousarray(x[b].reshape(C, N))
        in_maps.append(m)
    return in_maps


def kernel(**inputs) -> np.ndarray:
    gamma = float(np.asarray(inputs["gamma"]).reshape(-1)[0])
    key = gamma
    if key not in _cache:
        _cache.clear()
        _cache[key] = _build_program(gamma)
    nc = _cache[key]

    in_maps = _make_in_maps(inputs)
    res = bass_utils.run_bass_kernel_spmd(nc, in_maps, core_ids=list(range(NCORES)))
    out = np.stack([res.results[b]["out"].reshape(C, HH, WW) for b in range(B)])
    return out.astype(np.float32)
